# revision 4
# baseline (speedup 1.0000x reference)
"""CrossAttention Trainium2 Bass kernel — 8 cores, batch-per-core sharding.

Per core b: all H=8 heads of batch b.
  q = (q_data @ Wq + bq) * c^-0.5        -> computed transposed qT [hc, S]
  k = m_data @ Wk                        -> kT [hc, K]
  v = m_data @ Wv                        -> natural layout [K, h*v] (+ ones col per head)
  sT[k,q] = k @ qT  (per head, contraction c=32, PE row-strip packed)
  p = exp(sT + b)                        (bias shipped as per-row-int8, dequant on-chip)
  waT'[v+1, q] = sum_k v'[k, v+1] p[k, q]   (ones col -> softmax denominator in row 32)
  out[q, h, v] = waT[v, q].T * recip(den) * sigmoid(q_data @ Wg)

End-to-end latency through the axon tunnel is transfer-bound (~40MB/s up,
~17MB/s down), so the bias — the dominant input at B*H*S*K elements — is
shipped as uint8 with per-(h,k)-row affine scales (64MB instead of 128MB
fp16; measured output rel err 6.7e-3 vs the 2e-2 budget), the output is
returned fp16, and the jitted executable + donate-free output buffers are
cached across calls instead of being rebuilt per invocation.
"""
import numpy as np
from contextlib import ExitStack
from types import SimpleNamespace

import jax
import jax.numpy as jnp
from jax.experimental.shard_map import shard_map
from jax.sharding import Mesh, PartitionSpec, NamedSharding

import concourse.bass as bass
import concourse.tile as tile
from concourse import mybir
from concourse.bass2jax import (
    _bass_exec_p, install_neuronx_cc_hook, partition_id_tensor)
from concourse.masks import make_identity

F32 = mybir.dt.float32
F16 = mybir.dt.float16
U8 = mybir.dt.uint8

B, S, K, H, C, V, A = 8, 1024, 1024, 8, 32, 32, 256
HV = H * V            # 256
KEY_SCALE = C ** -0.5
N_CORES = 8
QT = S // 128         # 8 q tiles
KT = K // 128         # 8 k tiles


def _split_multi_waits(nc, max_waits=1):
    """walrus in this container allows only one semaphore wait per
    instruction; hoist extras onto same-engine nops inserted just before."""
    ctr = 0
    for fn in nc.m.functions:
        for blk in fn.blocks:
            insts = list(blk.instructions)
            out = []
            changed = False
            for inst in insts:
                si = inst.sync_info
                waits = list(si.on_wait) if (si is not None and si.on_wait) else []
                if len(waits) > max_waits:
                    changed = True
                    extra, keep = waits[:-max_waits], waits[-max_waits:]
                    for w in extra:
                        ctr += 1
                        nop = mybir.InstNoOp(
                            name=f"waitsplit_{ctr}",
                            engine=inst.engine,
                            ins=[],
                            outs=[],
                            sync_info=mybir.SyncInfo(on_wait=[w], on_update=[]),
                            bass_nofuse=True,
                        )
                        out.append(nop)
                    si.on_wait = keep
                out.append(inst)
            if changed:
                blk.set_instructions(out) if hasattr(blk, "set_instructions") else None
                if not hasattr(blk, "set_instructions"):
                    blk.instructions = out
    return ctr


def build():
    nc = bass.Bass()
    qT_d = nc.declare_dram_parameter("qT", [A, S], F16, isOutput=False)
    mT_d = nc.declare_dram_parameter("mT", [A, K], F16, isOutput=False)
    bu8_d = nc.declare_dram_parameter("bu8", [H, K, S], U8, isOutput=False)
    sclo_d = nc.declare_dram_parameter("sclo", [H, K, 2], F32, isOutput=False)
    wq_d = nc.declare_dram_parameter("wq", [A, HV], F16, isOutput=False)
    wk_d = nc.declare_dram_parameter("wk", [A, HV], F16, isOutput=False)
    wv_d = nc.declare_dram_parameter("wv", [A, HV], F16, isOutput=False)
    wg_d = nc.declare_dram_parameter("wg", [A, HV], F16, isOutput=False)
    bq_d = nc.declare_dram_parameter("bq", [HV], F32, isOutput=False)
    out_d = nc.declare_dram_parameter("out", [S, HV], F16, isOutput=True)

    with tile.TileContext(nc) as tc, ExitStack() as ctx:
        singles = ctx.enter_context(tc.tile_pool(name="singles", bufs=1))
        ub_pool = ctx.enter_context(tc.tile_pool(name="ub", bufs=4))
        t_pool = ctx.enter_context(tc.tile_pool(name="tt", bufs=2))
        p_pool = ctx.enter_context(tc.tile_pool(name="pp", bufs=4))
        wgs_pool = ctx.enter_context(tc.tile_pool(name="wgs", bufs=1))
        fin_pool = ctx.enter_context(tc.tile_pool(name="fin", bufs=4))
        ps_big = ctx.enter_context(tc.tile_pool(name="ps_big", bufs=2, space="PSUM"))
        ps_wa = ctx.enter_context(tc.tile_pool(name="ps_wa", bufs=1, space="PSUM"))
        ps_sm = ctx.enter_context(tc.tile_pool(name="ps_sm", bufs=2, space="PSUM"))

        # ---------- phase 0: load everything ----------
        qraw = singles.tile([128, 2, S], F16)       # [a-chunk part, chunk, q]
        mraw = singles.tile([128, 2, K], F16)
        for ac in range(2):
            nc.sync.dma_start(out=qraw[:, ac, :], in_=qT_d[ac * 128:(ac + 1) * 128, :])
            nc.sync.dma_start(out=mraw[:, ac, :], in_=mT_d[ac * 128:(ac + 1) * 128, :])
        wq_sb = singles.tile([128, 2, HV], F16)
        wk_sb = singles.tile([128, 2, HV], F16)
        wv_sb = singles.tile([128, 2, HV], F16)
        wg_sb = singles.tile([128, 2, HV], F16)
        for w_sb, w_d in ((wq_sb, wq_d), (wk_sb, wk_d), (wv_sb, wv_d), (wg_sb, wg_d)):
            for ac in range(2):
                nc.sync.dma_start(out=w_sb[:, ac, :], in_=w_d[ac * 128:(ac + 1) * 128, :])
        bq_sb = singles.tile([128, 2], F32)
        nc.sync.dma_start(out=bq_sb, in_=bq_d.rearrange("(h p) -> p h", p=128))
        # bias dequant scales: sc_all[p, h*KT+kt, 0:2] = sclo[h, kt*128+p, :]
        sc_all = singles.tile([128, H * KT, 2], F32)
        nc.sync.dma_start(
            out=sc_all,
            in_=sclo_d.rearrange("h (kt p) c -> p (h kt) c", p=128))
        ident = singles.tile([128, 128], F32)
        make_identity(nc, ident)

        # ---------- phase 1: projections ----------
        # gate[q, h*v] = sigmoid(q_data @ Wg), per q-tile (all heads packed)
        gate_sb = singles.tile([128, QT, HV], F32)
        for qt in range(QT):
            ps_g = ps_sm.tile([128, HV], F32, tag="ps_small")
            for ac in range(2):
                nc.tensor.matmul(ps_g, lhsT=qraw[:, ac, qt * 128:(qt + 1) * 128],
                                 rhs=wg_sb[:, ac, :], start=(ac == 0), stop=(ac == 1))
            nc.scalar.activation(gate_sb[:, qt, :], ps_g,
                                 mybir.ActivationFunctionType.Sigmoid)

        # qT_all / kT_all: [hc(4 heads), S] per half, scaled+biased q
        qT_sb = singles.tile([128, 2, S], F16)
        kT_sb = singles.tile([128, 2, K], F16)
        for half in range(2):
            for qh in range(2):
                ps_q = ps_big.tile([128, 512], F32, tag="ps_big")
                for ac in range(2):
                    nc.tensor.matmul(ps_q,
                                     lhsT=wq_sb[:, ac, half * 128:(half + 1) * 128],
                                     rhs=qraw[:, ac, qh * 512:(qh + 1) * 512],
                                     start=(ac == 0), stop=(ac == 1))
                nc.vector.tensor_scalar(
                    qT_sb[:, half, qh * 512:(qh + 1) * 512], ps_q,
                    KEY_SCALE, bq_sb[:, half:half + 1],
                    mybir.AluOpType.mult, mybir.AluOpType.add)
                ps_k = ps_big.tile([128, 512], F32, tag="ps_big")
                for ac in range(2):
                    nc.tensor.matmul(ps_k,
                                     lhsT=wk_sb[:, ac, half * 128:(half + 1) * 128],
                                     rhs=mraw[:, ac, qh * 512:(qh + 1) * 512],
                                     start=(ac == 0), stop=(ac == 1))
                nc.vector.tensor_copy(out=kT_sb[:, half, qh * 512:(qh + 1) * 512],
                                      in_=ps_k)

        # v natural layout + ones column: [k-tile part, h, v+1] fp16
        v_sb = singles.tile([128, KT, H, V + 1], F16)
        nc.gpsimd.memset(v_sb, 1.0)
        for kt in range(KT):
            ps_v = ps_sm.tile([128, HV], F32, tag="ps_small")
            for ac in range(2):
                nc.tensor.matmul(ps_v, lhsT=mraw[:, ac, kt * 128:(kt + 1) * 128],
                                 rhs=wv_sb[:, ac, :], start=(ac == 0), stop=(ac == 1))
            nc.vector.tensor_copy(
                out=v_sb[:, kt, :, 0:V],
                in_=ps_v.rearrange("p (h c) -> p h c", c=V))

        # ---------- phase 2: per-head attention + interleaved finalize ----------
        out_sb = singles.tile([128, QT, HV], F16)

        def finalize_head(h, ps_wa_t):
            wgt = wgs_pool.tile([33, S], F32, tag="wgt", bufs=2, name=f"wgt{h}")
            nc.vector.tensor_copy(out=wgt, in_=ps_wa_t)
            ps_t = ps_sm.tile([128, QT, V + 1], F32, tag="ps_small", name=f"ps_t{h}")
            for qt in range(QT):
                nc.tensor.transpose(ps_t[:, qt, :],
                                    wgt[:, qt * 128:(qt + 1) * 128],
                                    ident[0:33, 0:33])
            d_sb = fin_pool.tile([128, QT], F32, tag="d", name=f"d{h}")
            nc.vector.tensor_copy(out=d_sb, in_=ps_t[:, :, V])
            r_sb = fin_pool.tile([128, QT], F32, tag="r", name=f"r{h}")
            nc.vector.reciprocal(out=r_sb, in_=d_sb)
            rg_sb = fin_pool.tile([128, QT, V], F32, tag="rg", name=f"rg{h}")
            for qt in range(QT):
                nc.vector.tensor_scalar_mul(
                    rg_sb[:, qt, :],
                    gate_sb[:, qt, h * V:(h + 1) * V],
                    r_sb[:, qt:qt + 1])
            nc.vector.tensor_mul(
                out=out_sb.rearrange("p q (h c) -> p q h c", c=V)[:, :, h, :],
                in0=ps_t[:, :, 0:V],
                in1=rg_sb)

        pending = None  # (h, ps_wa_t) awaiting finalize
        for h in range(H):
            half, strip = h // 4, (h % 4) * 32
            ps_wa_t = ps_wa.tile([33, S], F32, tag="ps_wa", name=f"ps_wa{h}")
            for kt in range(KT):
                if kt == 2 and pending is not None:
                    finalize_head(*pending)
                    pending = None
                ps_s = ps_big.tile([128, S], F32, tag="ps_big")
                for qh in range(2):
                    nc.tensor.matmul(
                        ps_s[:, qh * 512:(qh + 1) * 512],
                        lhsT=kT_sb[strip:strip + 32, half, kt * 128:(kt + 1) * 128],
                        rhs=qT_sb[strip:strip + 32, half, qh * 512:(qh + 1) * 512],
                        start=True, stop=True,
                        tile_position=(strip, 0))
                ub = ub_pool.tile([128, S], U8, tag="ub")
                nc.sync.dma_start(out=ub, in_=bu8_d[h, kt * 128:(kt + 1) * 128, :])
                # t = ub * step + s   (vector), p = exp(t + lo)  (scalar)
                t = t_pool.tile([128, S], F32, tag="t")
                nc.vector.scalar_tensor_tensor(
                    out=t, in0=ub,
                    scalar=sc_all[:, h * KT + kt, 0:1],
                    in1=ps_s,
                    op0=mybir.AluOpType.mult, op1=mybir.AluOpType.add)
                p = p_pool.tile([128, S], F16, tag="p")
                nc.scalar.activation(p, t, mybir.ActivationFunctionType.Exp,
                                     bias=sc_all[:, h * KT + kt, 1:2])
                for qh in range(2):
                    nc.tensor.matmul(
                        ps_wa_t[:, qh * 512:(qh + 1) * 512],
                        lhsT=v_sb[:, kt, h, :],
                        rhs=p[:, qh * 512:(qh + 1) * 512],
                        start=(kt == 0), stop=(kt == KT - 1))
            pending = (h, ps_wa_t)
        finalize_head(*pending)

        # ---------- phase 3: store ----------
        for qt in range(QT):
            nc.sync.dma_start(out=out_d[qt * 128:(qt + 1) * 128, :],
                              in_=out_sb[:, qt, :])

    _split_multi_waits(nc)
    return nc


_RUNNER = None


def _get_runner():
    """Build the Bass module and a CACHED jitted shard_map executor.

    run_bass_kernel_spmd re-creates its jit closure per call, paying a
    full re-trace + re-lower each invocation; building it once here makes
    the steady-state call pure transfer + execute."""
    global _RUNNER
    if _RUNNER is not None:
        return _RUNNER

    nc = build()
    install_neuronx_cc_hook()
    partition_name = nc.partition_id_tensor.name if nc.partition_id_tensor else None

    in_names, out_names, out_avals = [], [], []
    for alloc in nc.m.functions[0].allocations:
        if not isinstance(alloc, mybir.MemoryLocationSet):
            continue
        name = alloc.memorylocations[0].name
        if alloc.kind == "ExternalInput":
            if name != partition_name:
                in_names.append(name)
        elif alloc.kind == "ExternalOutput":
            out_names.append(name)
            out_avals.append(jax.core.ShapedArray(
                tuple(alloc.tensor_shape), mybir.dt.np(alloc.dtype)))
    n_params = len(in_names)
    all_in_names = tuple(
        in_names + out_names + ([partition_name] if partition_name else []))

    def _body(*args):
        operands = list(args)
        if partition_name is not None:
            operands.append(partition_id_tensor())
        outs = _bass_exec_p.bind(
            *operands,
            out_avals=tuple(out_avals),
            in_names=all_in_names,
            out_names=tuple(out_names),
            lowering_input_output_aliases=(),
            sim_require_finite=True,
            sim_require_nnan=True,
            nc=nc,
        )
        return tuple(outs)

    devices = jax.devices()[:N_CORES]
    mesh = Mesh(np.asarray(devices), ("core",))
    n_outs = len(out_avals)
    in_specs = (PartitionSpec("core"),) * (n_params + n_outs)
    out_specs = (PartitionSpec("core"),) * n_outs
    sharded = jax.jit(
        shard_map(_body, mesh=mesh, in_specs=in_specs, out_specs=out_specs,
                  check_rep=False),
        keep_unused=True)

    # Output scratch buffers: the kernel writes every element of `out`, so
    # their contents never matter — keep device-resident zeros and reuse
    # them each call (no per-call host->device traffic, no donation).
    sh = NamedSharding(mesh, PartitionSpec("core"))
    out_scratch = [
        jax.device_put(
            np.zeros((N_CORES * av.shape[0], *av.shape[1:]), av.dtype), sh)
        for av in out_avals
    ]
    jax.block_until_ready(out_scratch)

    _RUNNER = SimpleNamespace(
        nc=nc, in_names=in_names, out_names=out_names, out_avals=out_avals,
        sharded=sharded, out_scratch=out_scratch, mesh=mesh)
    return _RUNNER


def _make_in_maps(q_data, m_data, batched_bias, query_w, query_b, key_w,
                  value_w, gating_w):
    """Host-side prep: global (concat-over-cores) input arrays, one per
    BIR parameter. Core c <- batch c. Bias is quantized to uint8 with
    per-(h,k)-row affine scales."""
    q_data = np.asarray(q_data, dtype=np.float32)
    m_data = np.asarray(m_data, dtype=np.float32)
    bias = np.asarray(batched_bias, dtype=np.float32)

    # bias [B,H,S,K] -> transposed [B,H,K,S], per-row uint8 affine quant
    bt = np.ascontiguousarray(bias.transpose(0, 1, 3, 2))
    lo = bt.min(axis=-1, keepdims=True)
    step = (bt.max(axis=-1, keepdims=True) - lo) / 254.0
    bu8 = np.clip(np.round((bt - lo) / step), 0, 254).astype(np.uint8)
    sclo = np.concatenate([step, lo], axis=-1).astype(np.float32)  # [B,H,K,2]

    wq = np.asarray(query_w, np.float32).reshape(A, HV).astype(np.float16)
    wk = np.asarray(key_w, np.float32).reshape(A, HV).astype(np.float16)
    wv = np.asarray(value_w, np.float32).reshape(A, HV).astype(np.float16)
    wg = np.asarray(gating_w, np.float32).reshape(A, HV).astype(np.float16)
    bq = (np.asarray(query_b, np.float32) * KEY_SCALE).reshape(HV)

    return {
        "qT": np.ascontiguousarray(q_data.transpose(0, 2, 1)).astype(np.float16)
              .reshape(B * A, S),
        "mT": np.ascontiguousarray(m_data.transpose(0, 2, 1)).astype(np.float16)
              .reshape(B * A, K),
        "bu8": np.ascontiguousarray(bu8).reshape(B * H, K, S),
        "sclo": np.ascontiguousarray(sclo).reshape(B * H, K, 2),
        "wq": np.tile(wq, (B, 1)), "wk": np.tile(wk, (B, 1)),
        "wv": np.tile(wv, (B, 1)), "wg": np.tile(wg, (B, 1)),
        "bq": np.tile(bq, B),
    }


def run_spmd(in_map, trace=False, **kw):
    r = _get_runner()
    if trace:
        # honest trace path: delegate to the framework runner (raises if
        # the axon NTFF hook is unavailable in this container)
        from concourse.bass_utils import run_bass_kernel_spmd
        per_core = []
        for c in range(N_CORES):
            m = {}
            for name in r.in_names:
                g = in_map[name]
                s0 = g.shape[0] // N_CORES
                m[name] = g[c * s0:(c + 1) * s0]
            per_core.append(m)
        return run_bass_kernel_spmd(r.nc, per_core, list(range(N_CORES)),
                                    trace=True, **kw)

    args = [in_map[name] for name in r.in_names] + r.out_scratch
    out_arrs = r.sharded(*args)
    out_np = np.asarray(out_arrs[0])  # [N_CORES*S, HV] fp16
    results = [{"out": out_np[c * S:(c + 1) * S]} for c in range(N_CORES)]
    return SimpleNamespace(results=results, exec_time_ns=None)


def kernel(q_data, m_data, batched_bias, query_w, query_b, key_w, value_w,
           gating_w):
    in_map = _make_in_maps(q_data, m_data, batched_bias, query_w, query_b,
                           key_w, value_w, gating_w)
    res = run_spmd(in_map)
    out = np.stack([res.results[b]["out"] for b in range(N_CORES)])
    return out.reshape(B, S, H, V).astype(np.float32)
